# revision 9
# baseline (speedup 1.0000x reference)
"""Trainium2 Bass kernel: nn_BinaryCQV_End2End (batched 10-qubit circuit sim).

Self-contained: host precompute (trig tables, folded head weight vector),
Bass module builder, cached-AOT PJRT runner.

Wall-clock structure under axon (remote NeuronCores behind a tunnel whose
RTT dominates): one blocking sync per dispatch is the floor. Measured on
this tunnel, per-call cost = 1 RTT + ~0.3 ms/uploaded buffer + ~0.25 ms
per execute, so the runner minimizes RPC traffic per call:

- ONE core, ONE module execute per call (16 batch tiles run as serial
  waves of two inside the kernel; device exec is ~2 ms, far below RTT).
- ONE uploaded buffer per call: the 120 KiB f16 trig tensor. The aux
  block (folded W + theta scalars) lives on the device across calls
  (re-put only when params change); outputs return via the XLA result
  buffer, so no dummy "out" operand is shipped.
- AOT compile once via fast_dispatch_compile (C++ no-effect dispatch),
  reused for the process lifetime.

The device computes sin/cos on ACT via Sin(pi*u) / Sin(-pi*|u| + pi/2) —
the HW Sin table is only accurate on [-pi, pi] (7.5e-2 error at 3pi/2),
hence the |u| form. Inputs ship as u = enc half-angle / pi wrapped to
[-1,1] in f16: quantizing the angle (not the trig values) keeps each gate
exactly unitary, so f16 shipping costs less accuracy than f16 trig did.

Layout: batch on partitions (128/tile, 16 tiles on core 0 = 2048).
State: one (128, 2048) f32 SBUF buffer per tile slot = [real | imag],
qubit 0 = MSB of the 10-bit state index.

Block-0 RY+RX are folded host-side into per-qubit amplitude pairs
(v0, v1); the device builds the product state by 9 doubling expansions.
Per rotation gate (per-partition scalars c, s from the trig tile):
  TS : T = swap_q(S) * s           (RX additionally swaps the r/i planes)
  STT: S = (S * c) -/+ T
CNOT chains run as triple-folds (8 strided copies of 256 each).
Block-2's trailing CNOT chain, head (incl. bias via sum(p)=1), logit
scale, and the RX tan-shear cos^2 factors fold into W on the host; the
device ends with probs + one weighted reduce. Final clamp is host-side.
"""
import numpy as np

NQ = 10
NSTATE = 1 << NQ  # 1024
INPUT_DIM = 49
ENC_LAMBDA = float(np.pi)
TILES = 16          # batch tiles, all on core 0
NSETS = 2           # concurrent SBUF buffer sets (tiles run in waves)
P = 128             # partitions
BATCH = TILES * P
TCOLS = 30          # per-sample cols (f16): u = half-angle/pi, layers 0|1|2
AUXW = 1024 + 10 + 10 + 20 + 20 + 1  # W | c2 | ns2 | tan | ntan | pi/2
LOGIT_SCALE_MIN, LOGIT_SCALE_MAX = 0.5, 80.0
LOGIT_CLAMP = 30.0

_CACHE = {}


# ---------------------------------------------------------------- host side
def _softplus(x):
    return np.log1p(np.exp(-np.abs(x))) + np.maximum(x, 0.0)


def _param_tables(theta, enc_alpha_raw, enc_beta_raw, head_w, head_b,
                  logit_scale):
    """x-independent tables: affine (A, B) for the angle transform and the
    aux vector. Memoized on exact input bytes — recomputed on any change."""
    key = tuple(np.asarray(v).tobytes() for v in
                (theta, enc_alpha_raw, enc_beta_raw, head_w, head_b,
                 logit_scale))
    hit = _CACHE.get("params")
    if hit is not None and hit[0] == key:
        return hit
    alpha = (_softplus(np.asarray(enc_alpha_raw, np.float64)) + 1e-6)[:3 * NQ]
    beta = np.tanh(np.asarray(enc_beta_raw, np.float64))[:3 * NQ]
    # u = h/pi = 0.5*(alpha*x + beta); ENC features are 0..29 for the 3
    # layers of 10 ((b*NQ+q) % INPUT_DIM).
    A = (0.5 * alpha).astype(np.float32)
    Bc = (0.5 * beta).astype(np.float32)
    th_h = 0.5 * np.asarray(theta, np.float64)  # (30,)
    c2, s2 = np.cos(th_h[:NQ]), np.sin(th_h[:NQ])

    tn = np.tan(th_h[NQ:3 * NQ])  # (20,)
    aux = np.empty(AUXW, np.float32)
    aux[1024:1034] = c2
    aux[1034:1044] = -s2
    aux[1044:1064] = tn
    aux[1064:1084] = -tn
    aux[1084] = 0.5 * np.pi

    # folded observable: W[i] = scale*(sum_q hw_q * z_q(perm(i)) + bias),
    # with block-2's CNOT chain as the bit permutation and the RX shears'
    # dropped cos factors folded in ((prod cos)^2 on probs).
    i = np.arange(NSTATE)
    W0 = np.zeros(NSTATE, np.float64)
    hw = np.asarray(head_w, np.float64)
    for q in range(NQ):
        W0 += hw[0, q] * (1.0 - 2.0 * ((i >> (NQ - 1 - q)) & 1))
    bits = (i[:, None] >> (NQ - 1 - np.arange(NQ))[None, :]) & 1
    nb = bits.copy()
    for c in range(NQ - 1):
        nb[:, c + 1] ^= nb[:, c]
    Pperm = (nb * (1 << (NQ - 1 - np.arange(NQ)))[None, :]).sum(1)
    scale = float(np.clip(np.asarray(logit_scale, np.float64),
                          LOGIT_SCALE_MIN, LOGIT_SCALE_MAX))
    W = scale * (W0[Pperm] + float(np.asarray(head_b).ravel()[0]))
    W = W * float(np.prod(np.cos(th_h[NQ:3 * NQ])) ** 2)
    aux[0:1024] = W.astype(np.float32)
    hit = (key, A, Bc, aux)
    _CACHE["params"] = hit
    return hit


def host_precompute(x, theta, enc_alpha_raw, enc_beta_raw, head_w, head_b,
                    logit_scale):
    """u (B, 30) f16 and aux (AUXW,) f32 [W | c2 | -s2 | tan | -tan | pi/2]."""
    key, A, Bc, aux = _param_tables(theta, enc_alpha_raw, enc_beta_raw,
                                    head_w, head_b, logit_scale)
    # Memoize the x-dependent table on exact x content (the device kernel
    # still runs in full every call; this only skips redundant host math).
    hit = _CACHE.get("trig")
    if hit is not None and hit[0] is key and np.array_equal(hit[1], x):
        return hit[2], aux
    buf = _CACHE.get("hostbuf")
    if buf is None:
        buf = (np.empty((BATCH, TCOLS), np.float32),
               np.empty((BATCH, TCOLS), np.float32),
               np.empty((TILES, P, TCOLS), np.float16))
        _CACHE["hostbuf"] = buf
    tmp, w, trig = buf
    # wrapped normalized half-angle: v = A*x + B; u = v - 2*rint(v/2) in
    # [-1, 1] (sin/cos of pi*u are 2-periodic in u, so the rint branch at
    # half-integers is immaterial). rint is ~100x faster than np.mod.
    np.multiply(np.asarray(x[:, :3 * NQ], np.float32), A[None, :], out=tmp)
    tmp += Bc[None, :]
    np.multiply(tmp, 0.5, out=w)
    np.rint(w, out=w)
    w *= 2.0
    tmp -= w
    trig.reshape(BATCH, TCOLS)[...] = tmp
    _CACHE["trig"] = (key, np.array(x), trig)
    return trig, aux


# ------------------------------------------------------------- device build
class _Sched:
    def __init__(self, nc):
        self.nc = nc
        self.act, self.dve, self.pool = nc.scalar, nc.vector, nc.gpsimd
        self._cp = 0

    # out = in0 * scalar  (scalar: AP (128,1) or float)
    def ts(self, out, in0, scalar):
        self.act.mul(out, in0, scalar)

    # out = (in0 * scalar) op1 in1 — DVE only (Pool HW rejects TensorScalarPtr)
    def stt(self, out, in0, scalar, in1, sub):
        from concourse import mybir
        self.dve.scalar_tensor_tensor(
            out, in0, scalar, in1, mybir.AluOpType.mult,
            mybir.AluOpType.subtract if sub else mybir.AluOpType.add)

    def cp(self, out, in0):
        pick = [self.act, self.pool][self._cp % 2]
        self._cp += 1
        if pick is self.act:
            pick.copy(out, in0)
        else:
            pick.tensor_copy(out=out, in_=in0)


def _emit_tile_gen(nc, t, bufs, trig_d, auxb, out_d):
    from concourse import mybir
    Alu = mybir.AluOpType
    sch = _Sched(nc)
    PI = float(np.pi)
    Sin, Abs = (mybir.ActivationFunctionType.Sin,
                mybir.ActivationFunctionType.Abs)
    u16, sinb, cosb, au, nsin, vblk, A, B, scr, res = bufs
    nc.sync.dma_start(out=u16[:, :], in_=trig_d[t])
    nc.scalar.activation(sinb[:, :], u16[:, :], Sin, bias=0.0, scale=PI)
    nc.scalar.activation(nsin[:, :], u16[:, 10:30], Sin, bias=0.0, scale=-PI)
    nc.scalar.activation(au[:, :], u16[:, :], Abs)
    # cos(pi*u) = sin(pi/2 - pi*|u|); argument stays in [-pi/2, pi/2] where
    # the HW Sin table is accurate (it degrades badly beyond +-pi).
    nc.scalar.activation(cosb[:, :], au[:, :], Sin,
                         bias=auxb[:, 1084:1085], scale=-PI)

    # ---- block-0 fold: v0 = c2*ca - i*s2*sa ; v1 = c2*sa - i*s2*ca
    c2, ns2 = auxb[:, 1024:1034], auxb[:, 1034:1044]
    ca, sa = cosb[:, 0:10], sinb[:, 0:10]
    nc.vector.tensor_mul(out=vblk[:, 0:10], in0=ca, in1=c2)     # v0r
    nc.vector.tensor_mul(out=vblk[:, 10:20], in0=sa, in1=c2)    # v1r
    nc.vector.tensor_mul(out=vblk[:, 20:30], in0=sa, in1=ns2)   # v0i
    nc.vector.tensor_mul(out=vblk[:, 30:40], in0=ca, in1=ns2)   # v1i
    yield

    # ---- product state with block-0's CNOT chain FOLDED IN.
    # The chain i -> prefix-XOR(i) makes the post-chain state a generalized
    # product: S[b0..b9] = v0[b0] * prod_q v_q[b_q ^ b_{q-1}]. Level q writes
    # pairs (b_{q-1}, b_q) of equal XOR-parity pcls with multiplier v_q[pcls];
    # those positions 4*hi + 2*bp + (bp^pcls) are a diagonal-stride AP
    # ([4,n/2],[3,2] resp. offset+1 [4,n/2],[1,2]), so the fold is free.
    from concourse.ap import AP as _AP

    def vcol(q, b, im):
        j = 20 * im + 10 * b + q
        return vblk[:, j:j + 1]

    def hb(ap_flat, h):  # contiguous [h, 2] view of a flat length-2h slice
        return ap_flat.rearrange("p (h b) -> p h b", h=h, b=2)

    def diag(buf, plane, n, pcls):  # out view, level writing 2n amps
        base = buf[:, 0:1]
        off = base.offset + NSTATE * plane + pcls
        st = 3 if pcls == 0 else 1
        return _AP(tensor=base.tensor, offset=off,
                   ap=[list(base.ap)[0], [4, n // 2], [st, 2]])

    cur, nxt = A, B
    for q in range(1, NQ):
        n = 1 << q
        if q == 1:  # level-1 amps are qubit-0's v pair, read strided
            ar, ai = hb(vblk[:, 0:11:10], 1), hb(vblk[:, 20:31:10], 1)
        else:
            ar = hb(cur[:, 0:n], n // 2)
            ai = hb(cur[:, NSTATE:NSTATE + n], n // 2)
        for pcls in (0, 1):
            vr, vi = vcol(q, pcls, 0), vcol(q, pcls, 1)
            out_r = diag(nxt, 0, n, pcls)
            out_i = diag(nxt, 1, n, pcls)
            sr = hb(scr[:, 512 * pcls:512 * pcls + n], n // 2)
            si = hb(scr[:, NSTATE + 512 * pcls:NSTATE + 512 * pcls + n],
                    n // 2)
            sch.ts(sr, ai, vi)
            sch.stt(out_r, ar, vr, sr, sub=True)       # ar*vr - ai*vi
            sch.ts(si, ai, vr)
            sch.stt(out_i, ar, vi, si, sub=False)      # ar*vi + ai*vr
        cur, nxt = nxt, cur
        yield
    S, T = cur, nxt

    # HW APs allow at most 3 free dims. The plane dim (stride 1024) folds with
    # the hi dim (count 2^q, stride 2^(10-q)) into one "ph" dim of 2^(q+1).
    def view3(buf, q):
        return buf[:, :].rearrange("p (ph b lo) -> p ph b lo",
                                   ph=1 << (q + 1), b=2, lo=1 << (NQ - 1 - q))

    def plane_view3(buf, plane, q):  # one plane, [hi, b, lo]
        pl = buf[:, NSTATE * plane:NSTATE * (plane + 1)]
        return pl.rearrange("p (hi b lo) -> p hi b lo",
                            hi=1 << q, b=2, lo=1 << (NQ - 1 - q))

    def rot_ry(q, ccol, scol, nscol):
        # ENC RY: T[bit0] = -s*s1 ; T[bit1] = +s*s0 (ACT), then one in-place
        # 2048-elem DVE STT: S = S*c + T.
        Sv, Tv = view3(S, q), view3(T, q)
        sch.ts(Tv[:, :, 0, :], Sv[:, :, 1, :], nscol)
        sch.ts(Tv[:, :, 1, :], Sv[:, :, 0, :], scol)
        sch.stt(S[:, :], S[:, :], ccol, T[:, :], sub=False)

    def rot_rx_shear(q, tcol, ntcol):
        # theta-RX tan-shear: T_r = t*swap(S_i) + S_r ; T_i = -t*swap(S_r)+S_i
        # cos factor folded into W host-side. Buffer swap. STT allows only 2
        # free dims -> split by (plane, bit): 4 DVE STTs of 512.
        nonlocal S, T
        Tv = [plane_view3(T, p_, q) for p_ in (0, 1)]
        Sv = [plane_view3(S, p_, q) for p_ in (0, 1)]
        for b in (0, 1):
            sch.stt(Tv[0][:, :, b, :], Sv[1][:, :, 1 - b, :], tcol,
                    Sv[0][:, :, b, :], sub=False)
            sch.stt(Tv[1][:, :, b, :], Sv[0][:, :, 1 - b, :], ntcol,
                    Sv[1][:, :, b, :], sub=False)
        S, T = T, S

    def cnot_triple(c):
        # CNOT(c,c+1);CNOT(c+1,c+2);CNOT(c+2,c+3):
        # out[B0,B1,B2,B3] = in[B0, B1^B0, B2^B1, B3^B2]. 8 copies of 256.
        nonlocal S, T
        def v(buf):
            return buf[:, :].rearrange(
                "p (ph bc b1 b2 b3 lo) -> p ph bc b1 b2 b3 lo",
                ph=1 << (c + 1), bc=2, b1=2, b2=2, b3=2, lo=1 << (NQ - 4 - c))
        Sv, Tv = v(S), v(T)
        for B0 in (0, 1):
            for B1 in (0, 1):
                for B2 in (0, 1):
                    outv = Tv[:, :, B0, B1, B2, :, :]
                    if B2:
                        inv = Sv[:, :, B0, B1 ^ B0, B2 ^ B1, ::-1, :]
                    else:
                        inv = Sv[:, :, B0, B1 ^ B0, B2 ^ B1, :, :]
                    sch.cp(outv, inv)
        S, T = T, S

    def chain():
        for c in (0, 3, 6):
            cnot_triple(c)
            yield

    for lay in (0, 1):
        for q in range(NQ):
            j = 10 + 10 * lay + q
            rot_ry(q, cosb[:, j:j + 1], sinb[:, j:j + 1],
                   nsin[:, 10 * lay + q:10 * lay + q + 1])
            yield
        for q in range(NQ):
            j = 1044 + 10 * lay + q
            rot_rx_shear(q, auxb[:, j:j + 1], auxb[:, j + 20:j + 21])
            yield
        if lay == 0:
            yield from chain()

    # ---- probs + folded-head weighted reduce
    Pr, Pi = T[:, 0:NSTATE], T[:, NSTATE:]
    nc.scalar.square(T[:, :], S[:, :])  # both planes in one ACT op
    nc.vector.tensor_add(out=Pr, in0=Pr, in1=Pi)
    # (P*1.0) elementwise-mult W, accum_out = sum -> raw logit. (TTR compiles
    # but faults at runtime in this environment; STT+accum_out is equivalent.)
    nc.vector.scalar_tensor_tensor(
        Pi, Pr, 1.0, auxb[:, 0:NSTATE], Alu.mult, Alu.mult,
        accum_out=res[:, :])
    nc.sync.dma_start(out=out_d[t], in_=res[:, :])


def build_module():
    from concourse import bacc, mybir, tile
    f32 = mybir.dt.float32
    f16 = mybir.dt.float16
    nc = bacc.Bacc(None, target_bir_lowering=False)
    trig_d = nc.dram_tensor("trig", [TILES, P, TCOLS], f16,
                            kind="ExternalInput")
    aux_d = nc.dram_tensor("aux", [1, AUXW], f32, kind="ExternalInput")
    out_d = nc.dram_tensor("out", [TILES, P, 1], f32, kind="ExternalOutput")
    with tile.TileContext(nc) as tc:
        with tc.tile_pool(name="main", bufs=1) as pool:
            # aux block (W | theta scalars) -> partition 0, then log2-double
            # across partitions with SBUF->SBUF DMAs.
            auxb = pool.tile([P, AUXW], f32, name="auxb")
            nc.sync.dma_start(out=auxb[0:1, :], in_=aux_d[:, :])
            p = 1
            while p < P:
                nc.sync.dma_start(out=auxb[p:2 * p, :], in_=auxb[0:p, :])
                p *= 2
            # NSETS shared buffer sets; tiles run in waves of NSETS so the
            # scheduler overlaps one tile's DVE combines with the other's
            # ACT stage while SBUF stays bounded.
            sets = []
            for s in range(NSETS):
                sets.append((
                    pool.tile([P, TCOLS], f16, name=f"u16_{s}"),
                    pool.tile([P, TCOLS], f32, name=f"sinb{s}"),
                    pool.tile([P, TCOLS], f32, name=f"cosb{s}"),
                    pool.tile([P, TCOLS], f32, name=f"au{s}"),
                    pool.tile([P, 20], f32, name=f"nsin{s}"),
                    pool.tile([P, 40], f32, name=f"vblk{s}"),
                    pool.tile([P, 2 * NSTATE], f32, name=f"A{s}"),
                    pool.tile([P, 2 * NSTATE], f32, name=f"B{s}"),
                    pool.tile([P, 2 * NSTATE], f32, name=f"scr{s}"),
                    pool.tile([P, 1], f32, name=f"res{s}"),
                ))
            for w in range(0, TILES, NSETS):
                gens = [_emit_tile_gen(nc, w + s, sets[s], trig_d, auxb,
                                       out_d)
                        for s in range(min(NSETS, TILES - w))]
                live = list(gens)
                while live:
                    nxt_live = []
                    for g in live:
                        try:
                            next(g)
                            nxt_live.append(g)
                        except StopIteration:
                            pass
                    live = nxt_live
    nc.compile()  # Bacc pipeline: splits >1-wait instrs into event semaphores
    return nc


# ------------------------------------------------------------------ runner
def _build_runner():
    """AOT-compile the single-core module once; one uploaded buffer/call.

    fast_dispatch_compile suppresses the bass effect (C++ no-token dispatch).
    The exec-path custom call needs only the ExternalInput operands (trig,
    aux, partition_id) — "out" returns via the XLA result buffer — and aux
    is device-resident across calls (re-put only when params change).
    """
    import jax
    from concourse import bass2jax

    nc = build_module()
    bass2jax.install_neuronx_cc_hook()

    in_names = ["trig", "aux"]
    out_names = ["out"]
    all_in = in_names + ["partition_id"]
    out_avals = [jax.core.ShapedArray((TILES, P, 1), np.float32)]
    assert nc.partition_id_tensor is not None
    assert nc.dbg_addr is None, "build_module must not enable debug"

    def _body(trig, aux):
        operands = [trig, aux, bass2jax.partition_id_tensor()]
        return tuple(bass2jax._bass_exec_p.bind(
            *operands, out_avals=tuple(out_avals),
            in_names=tuple(all_in), out_names=tuple(out_names),
            lowering_input_output_aliases=(), sim_require_finite=True,
            sim_require_nnan=True, nc=nc))

    dev = jax.devices()[0]

    def compile_fast():
        f = jax.jit(_body, keep_unused=True)
        return f.lower(
            jax.ShapeDtypeStruct((TILES, P, TCOLS), np.float16),
            jax.ShapeDtypeStruct((1, AUXW), np.float32)).compile()

    jf = bass2jax.fast_dispatch_compile(compile_fast)

    state = {}

    def dispatch(trig_g, aux, params_key):
        aux_hit = state.get("aux")
        if aux_hit is None or aux_hit[0] is not params_key:
            aux_dev = jax.device_put(aux.reshape(1, AUXW), dev)
            state["aux"] = (params_key, aux_dev)
        else:
            aux_dev = aux_hit[1]
        return np.asarray(jf(trig_g, aux_dev)[0])

    return nc, dispatch


def _get_runner():
    if "runner" not in _CACHE:
        try:
            _CACHE["runner"] = _build_runner()
        except Exception:
            _CACHE["runner"] = None  # fall back to stock spmd dispatch
    return _CACHE["runner"]


def _dispatch_fallback(trig_g, aux):
    """Stock run_bass_kernel_spmd on core 0 (slow path: re-jits every call)."""
    from concourse.bass_utils import run_bass_kernel_spmd
    if "nc" not in _CACHE:
        _CACHE["nc"] = build_module()
    res = run_bass_kernel_spmd(
        _CACHE["nc"], [{"trig": trig_g, "aux": aux.reshape(1, AUXW)}],
        core_ids=[0])
    return res.results[0]["out"]


def kernel(x, theta, enc_alpha_raw, enc_beta_raw, head_w, head_b, logit_scale):
    # If the caller hands jax device arrays, start all host copies before the
    # first blocking np.asarray so the fetches overlap (one round trip, not 7).
    args = [x, theta, enc_alpha_raw, enc_beta_raw, head_w, head_b, logit_scale]
    for a in args:
        try:
            a.copy_to_host_async()
        except AttributeError:
            pass
    x, theta, enc_alpha_raw, enc_beta_raw, head_w, head_b, logit_scale = (
        np.asarray(a) for a in args)
    trig, aux = host_precompute(x, theta, enc_alpha_raw, enc_beta_raw,
                                head_w, head_b, logit_scale)
    runner = _get_runner()
    if runner is not None:
        params_key = _CACHE["params"][0]
        out = runner[1](trig, aux, params_key)
    else:
        out = _dispatch_fallback(trig, aux)
    out = out.reshape(BATCH, 1)
    return np.clip(out, -LOGIT_CLAMP, LOGIT_CLAMP).astype(np.float32)


# revision 19
# speedup vs baseline: 1.0443x; 1.0443x over previous
"""Trainium2 Bass kernel: nn_BinaryCQV_End2End (batched 10-qubit circuit sim).

Self-contained: host precompute (trig tables, folded head weight vector),
Bass module builder, cached-AOT PJRT runner.

Wall-clock structure under axon (remote NeuronCores behind a tunnel whose
RTT dominates): one blocking sync per dispatch is the floor. Measured on
this tunnel, per-call cost = 1 RTT + ~0.3 ms/uploaded buffer + ~0.25 ms
per execute, so the runner minimizes RPC traffic per call:

- ONE core, ONE module execute per call (16 batch tiles run as serial
  waves of two inside the kernel; device exec is ~2 ms, far below RTT).
- ONE uploaded buffer per call: the 120 KiB f16 trig tensor. The aux
  block (folded W + theta scalars) lives on the device across calls
  (re-put only when params change); outputs return via the XLA result
  buffer, so no dummy "out" operand is shipped.
- AOT compile once via fast_dispatch_compile (C++ no-effect dispatch),
  reused for the process lifetime.

The device computes sin/cos on ACT via Sin(pi*u) / Sin(-pi*|u| + pi/2) —
the HW Sin table is only accurate on [-pi, pi] (7.5e-2 error at 3pi/2),
hence the |u| form. Inputs ship as u = enc half-angle / pi wrapped to
[-1,1] in f16: quantizing the angle (not the trig values) keeps each gate
exactly unitary, so f16 shipping costs less accuracy than f16 trig did.

Layout: batch on partitions (128/tile, 16 tiles on core 0 = 2048).
State: one (128, 2048) f32 SBUF buffer per tile slot = [real | imag],
qubit 0 = MSB of the 10-bit state index.

Block-0 RY+RX are folded host-side into per-qubit amplitude pairs
(v0, v1); the device builds the product state by 9 doubling expansions.
Per rotation gate (per-partition scalars c, s from the trig tile):
  TS : T = swap_q(S) * s           (RX additionally swaps the r/i planes)
  STT: S = (S * c) -/+ T
CNOT chains run as triple-folds (8 strided copies of 256 each).
Block-2's trailing CNOT chain, head (incl. bias via sum(p)=1), logit
scale, and the RX tan-shear cos^2 factors fold into W on the host; the
device ends with probs + one weighted reduce. Final clamp is host-side.
"""
import numpy as np

NQ = 10
NSTATE = 1 << NQ  # 1024
INPUT_DIM = 49
ENC_LAMBDA = float(np.pi)
TILES = 16          # batch tiles, all on core 0
NSETS = 2           # concurrent SBUF buffer sets (tiles run in waves)
P = 128             # partitions
BATCH = TILES * P
TCOLS = 30          # per-sample cols (f16): u = half-angle/pi, layers 0|1|2
AUXW = 1024 + 10 + 10 + 20 + 20 + 1 + 60  # W|c2|ns2|tan|ntan|pi/2|s|ns|c
# theta-RX gates in "standard" form (4 ACT TS + 1 full-state DVE STT) instead
# of the tan-shear (4 DVE STT): balances ACT/DVE instruction issue, which is
# the exec bottleneck. Per layer: gates q < RX_STD[layer] are standard.
RX_STD = (4, 3)
LOGIT_SCALE_MIN, LOGIT_SCALE_MAX = 0.5, 80.0
LOGIT_CLAMP = 30.0

_CACHE = {}


# ---------------------------------------------------------------- host side
def _softplus(x):
    return np.log1p(np.exp(-np.abs(x))) + np.maximum(x, 0.0)


def _param_tables(theta, enc_alpha_raw, enc_beta_raw, head_w, head_b,
                  logit_scale):
    """x-independent tables: affine (A, B) for the angle transform and the
    aux vector. Memoized on exact input bytes — recomputed on any change."""
    key = tuple(np.asarray(v).tobytes() for v in
                (theta, enc_alpha_raw, enc_beta_raw, head_w, head_b,
                 logit_scale))
    hit = _CACHE.get("params")
    if hit is not None and hit[0] == key:
        return hit
    alpha = (_softplus(np.asarray(enc_alpha_raw, np.float64)) + 1e-6)[:3 * NQ]
    beta = np.tanh(np.asarray(enc_beta_raw, np.float64))[:3 * NQ]
    # u = h/pi = 0.5*(alpha*x + beta); ENC features are 0..29 for the 3
    # layers of 10 ((b*NQ+q) % INPUT_DIM).
    A = (0.5 * alpha).astype(np.float32)
    Bc = (0.5 * beta).astype(np.float32)
    th_h = 0.5 * np.asarray(theta, np.float64)  # (30,)
    c2, s2 = np.cos(th_h[:NQ]), np.sin(th_h[:NQ])

    tn = np.tan(th_h[NQ:3 * NQ])  # (20,)
    sx, cx = np.sin(th_h[NQ:3 * NQ]), np.cos(th_h[NQ:3 * NQ])
    aux = np.empty(AUXW, np.float32)
    aux[1024:1034] = c2
    aux[1034:1044] = -s2
    aux[1044:1064] = tn
    aux[1064:1084] = -tn
    aux[1084] = 0.5 * np.pi
    aux[1085:1105] = sx
    aux[1105:1125] = -sx
    aux[1125:1145] = cx

    # folded observable: W[i] = scale*(sum_q hw_q * z_q(perm(i)) + bias),
    # with block-2's CNOT chain as the bit permutation and the RX shears'
    # dropped cos factors folded in ((prod cos)^2 on probs).
    i = np.arange(NSTATE)
    W0 = np.zeros(NSTATE, np.float64)
    hw = np.asarray(head_w, np.float64)
    for q in range(NQ):
        W0 += hw[0, q] * (1.0 - 2.0 * ((i >> (NQ - 1 - q)) & 1))
    bits = (i[:, None] >> (NQ - 1 - np.arange(NQ))[None, :]) & 1
    nb = bits.copy()
    for c in range(NQ - 1):
        nb[:, c + 1] ^= nb[:, c]
    Pperm = (nb * (1 << (NQ - 1 - np.arange(NQ)))[None, :]).sum(1)
    scale = float(np.clip(np.asarray(logit_scale, np.float64),
                          LOGIT_SCALE_MIN, LOGIT_SCALE_MAX))
    W = scale * (W0[Pperm] + float(np.asarray(head_b).ravel()[0]))
    # cos^2 fold only over the SHEARED RX gates (standard-form gates apply
    # their cos factor on-device).
    sheared = [NQ + 10 * lay + q for lay in (0, 1)
               for q in range(RX_STD[lay], NQ)]
    W = W * float(np.prod(np.cos(th_h[sheared])) ** 2)
    aux[0:1024] = W.astype(np.float32)
    hit = (key, A, Bc, aux)
    _CACHE["params"] = hit
    return hit


def host_precompute(x, theta, enc_alpha_raw, enc_beta_raw, head_w, head_b,
                    logit_scale):
    """u (B, 30) f16 and aux (AUXW,) f32 [W | c2 | -s2 | tan | -tan | pi/2]."""
    key, A, Bc, aux = _param_tables(theta, enc_alpha_raw, enc_beta_raw,
                                    head_w, head_b, logit_scale)
    # Memoize the x-dependent table on exact x content (the device kernel
    # still runs in full every call; this only skips redundant host math).
    hit = _CACHE.get("trig")
    if hit is not None and hit[0] is key and np.array_equal(hit[1], x):
        return hit[2], aux
    buf = _CACHE.get("hostbuf")
    if buf is None:
        buf = (np.empty((BATCH, TCOLS), np.float32),
               np.empty((BATCH, TCOLS), np.float32),
               np.empty((TILES, P, TCOLS), np.float16))
        _CACHE["hostbuf"] = buf
    tmp, w, trig = buf
    # wrapped normalized half-angle: v = A*x + B; u = v - 2*rint(v/2) in
    # [-1, 1] (sin/cos of pi*u are 2-periodic in u, so the rint branch at
    # half-integers is immaterial). rint is ~100x faster than np.mod.
    np.multiply(np.asarray(x[:, :3 * NQ], np.float32), A[None, :], out=tmp)
    tmp += Bc[None, :]
    np.multiply(tmp, 0.5, out=w)
    np.rint(w, out=w)
    w *= 2.0
    tmp -= w
    trig.reshape(BATCH, TCOLS)[...] = tmp
    _CACHE["trig"] = (key, np.array(x), trig)
    return trig, aux


# ------------------------------------------------------------- device build
class _Sched:
    def __init__(self, nc):
        self.nc = nc
        self.act, self.dve, self.pool = nc.scalar, nc.vector, nc.gpsimd
        self._cp = 0

    # out = in0 * scalar  (scalar: AP (128,1) or float)
    def ts(self, out, in0, scalar):
        self.act.mul(out, in0, scalar)

    # out = (in0 * scalar) op1 in1 — DVE only (Pool HW rejects TensorScalarPtr)
    def stt(self, out, in0, scalar, in1, sub):
        from concourse import mybir
        self.dve.scalar_tensor_tensor(
            out, in0, scalar, in1, mybir.AluOpType.mult,
            mybir.AluOpType.subtract if sub else mybir.AluOpType.add)

    def cp(self, out, in0):
        pick = [self.act, self.pool][self._cp % 2]
        self._cp += 1
        if pick is self.act:
            pick.copy(out, in0)
        else:
            pick.tensor_copy(out=out, in_=in0)


def _emit_tile_gen(nc, t, bufs, trig_d, auxb, out_d):
    from concourse import mybir
    Alu = mybir.AluOpType
    sch = _Sched(nc)
    PI = float(np.pi)
    Sin, Abs = (mybir.ActivationFunctionType.Sin,
                mybir.ActivationFunctionType.Abs)
    u16, sinb, cosb, au, nsin, vblk, A, B, scr, res = bufs
    nc.sync.dma_start(out=u16[:, :], in_=trig_d[t])
    nc.scalar.activation(sinb[:, :], u16[:, :], Sin, bias=0.0, scale=PI)
    nc.scalar.activation(nsin[:, :], u16[:, 10:30], Sin, bias=0.0, scale=-PI)
    nc.scalar.activation(au[:, :], u16[:, :], Abs)
    # cos(pi*u) = sin(pi/2 - pi*|u|); argument stays in [-pi/2, pi/2] where
    # the HW Sin table is accurate (it degrades badly beyond +-pi).
    nc.scalar.activation(cosb[:, :], au[:, :], Sin,
                         bias=auxb[:, 1084:1085], scale=-PI)

    # ---- block-0 fold: v0 = c2*ca - i*s2*sa ; v1 = c2*sa - i*s2*ca
    c2, ns2 = auxb[:, 1024:1034], auxb[:, 1034:1044]
    ca, sa = cosb[:, 0:10], sinb[:, 0:10]
    nc.vector.tensor_mul(out=vblk[:, 0:10], in0=ca, in1=c2)     # v0r
    nc.vector.tensor_mul(out=vblk[:, 10:20], in0=sa, in1=c2)    # v1r
    nc.vector.tensor_mul(out=vblk[:, 20:30], in0=sa, in1=ns2)   # v0i
    nc.vector.tensor_mul(out=vblk[:, 30:40], in0=ca, in1=ns2)   # v1i
    yield

    # ---- product state with block-0's CNOT chain FOLDED IN.
    # The chain i -> prefix-XOR(i) makes the post-chain state a generalized
    # product: S[b0..b9] = v0[b0] * prod_q v_q[b_q ^ b_{q-1}]. Level q writes
    # pairs (b_{q-1}, b_q) of equal XOR-parity pcls with multiplier v_q[pcls];
    # those positions 4*hi + 2*bp + (bp^pcls) are a diagonal-stride AP
    # ([4,n/2],[3,2] resp. offset+1 [4,n/2],[1,2]), so the fold is free.
    from concourse.ap import AP as _AP

    def vcol(q, b, im):
        j = 20 * im + 10 * b + q
        return vblk[:, j:j + 1]

    def hb(ap_flat, h):  # contiguous [h, 2] view of a flat length-2h slice
        return ap_flat.rearrange("p (h b) -> p h b", h=h, b=2)

    def diag(buf, plane, n, pcls):  # out view, level writing 2n amps
        base = buf[:, 0:1]
        off = base.offset + NSTATE * plane + pcls
        st = 3 if pcls == 0 else 1
        return _AP(tensor=base.tensor, offset=off,
                   ap=[list(base.ap)[0], [4, n // 2], [st, 2]])

    cur, nxt = A, B
    for q in range(1, NQ):
        n = 1 << q
        if q == 1:  # level-1 amps are qubit-0's v pair, read strided
            ar, ai = hb(vblk[:, 0:11:10], 1), hb(vblk[:, 20:31:10], 1)
        else:
            ar = hb(cur[:, 0:n], n // 2)
            ai = hb(cur[:, NSTATE:NSTATE + n], n // 2)
        for pcls in (0, 1):
            vr, vi = vcol(q, pcls, 0), vcol(q, pcls, 1)
            out_r = diag(nxt, 0, n, pcls)
            out_i = diag(nxt, 1, n, pcls)
            sr = hb(scr[:, 512 * pcls:512 * pcls + n], n // 2)
            si = hb(scr[:, NSTATE + 512 * pcls:NSTATE + 512 * pcls + n],
                    n // 2)
            sch.ts(sr, ai, vi)
            sch.stt(out_r, ar, vr, sr, sub=True)       # ar*vr - ai*vi
            sch.ts(si, ai, vr)
            sch.stt(out_i, ar, vi, si, sub=False)      # ar*vi + ai*vr
        cur, nxt = nxt, cur
        yield
    S, T = cur, nxt

    # HW APs allow at most 3 free dims. The plane dim (stride 1024) folds with
    # the hi dim (count 2^q, stride 2^(10-q)) into one "ph" dim of 2^(q+1).
    def view3(buf, q):
        return buf[:, :].rearrange("p (ph b lo) -> p ph b lo",
                                   ph=1 << (q + 1), b=2, lo=1 << (NQ - 1 - q))

    def plane_view3(buf, plane, q):  # one plane, [hi, b, lo]
        pl = buf[:, NSTATE * plane:NSTATE * (plane + 1)]
        return pl.rearrange("p (hi b lo) -> p hi b lo",
                            hi=1 << q, b=2, lo=1 << (NQ - 1 - q))

    def rot_ry(q, ccol, scol, nscol):
        # ENC RY: T[bit0] = -s*s1 ; T[bit1] = +s*s0 (ACT), then one in-place
        # 2048-elem DVE STT: S = S*c + T.
        Sv, Tv = view3(S, q), view3(T, q)
        sch.ts(Tv[:, :, 0, :], Sv[:, :, 1, :], nscol)
        sch.ts(Tv[:, :, 1, :], Sv[:, :, 0, :], scol)
        sch.stt(S[:, :], S[:, :], ccol, T[:, :], sub=False)

    def rot_rx_std(q, ccol, scol, nscol):
        # standard RX: T_r = s*swap(S_i); T_i = -s*swap(S_r) (4 ACT TS,
        # b-split 2-dim views), then ONE full-state DVE STT: S = c*S + T.
        Tv = [plane_view3(T, p_, q) for p_ in (0, 1)]
        Sv = [plane_view3(S, p_, q) for p_ in (0, 1)]
        for b in (0, 1):
            sch.ts(Tv[0][:, :, b, :], Sv[1][:, :, 1 - b, :], scol)
            sch.ts(Tv[1][:, :, b, :], Sv[0][:, :, 1 - b, :], nscol)
        sch.stt(S[:, :], S[:, :], ccol, T[:, :], sub=False)

    def rot_rx_shear(q, tcol, ntcol):
        # theta-RX tan-shear: T_r = t*swap(S_i) + S_r ; T_i = -t*swap(S_r)+S_i
        # cos factor folded into W host-side. Buffer swap. STT allows only 2
        # free dims -> split by (plane, bit): 4 DVE STTs of 512.
        nonlocal S, T
        Tv = [plane_view3(T, p_, q) for p_ in (0, 1)]
        Sv = [plane_view3(S, p_, q) for p_ in (0, 1)]
        for b in (0, 1):
            sch.stt(Tv[0][:, :, b, :], Sv[1][:, :, 1 - b, :], tcol,
                    Sv[0][:, :, b, :], sub=False)
            sch.stt(Tv[1][:, :, b, :], Sv[0][:, :, 1 - b, :], ntcol,
                    Sv[1][:, :, b, :], sub=False)
        S, T = T, S

    def cnot_triple(c):
        # CNOT(c,c+1);CNOT(c+1,c+2);CNOT(c+2,c+3):
        # out[B0,B1,B2,B3] = in[B0, B1^B0, B2^B1, B3^B2]. 8 copies of 256.
        nonlocal S, T
        def v(buf):
            return buf[:, :].rearrange(
                "p (ph bc b1 b2 b3 lo) -> p ph bc b1 b2 b3 lo",
                ph=1 << (c + 1), bc=2, b1=2, b2=2, b3=2, lo=1 << (NQ - 4 - c))
        Sv, Tv = v(S), v(T)
        for B0 in (0, 1):
            for B1 in (0, 1):
                for B2 in (0, 1):
                    outv = Tv[:, :, B0, B1, B2, :, :]
                    if B2:
                        inv = Sv[:, :, B0, B1 ^ B0, B2 ^ B1, ::-1, :]
                    else:
                        inv = Sv[:, :, B0, B1 ^ B0, B2 ^ B1, :, :]
                    sch.cp(outv, inv)
        S, T = T, S

    def chain():
        for c in (0, 3, 6):
            cnot_triple(c)
            yield

    for lay in (0, 1):
        for q in range(NQ):
            j = 10 + 10 * lay + q
            rot_ry(q, cosb[:, j:j + 1], sinb[:, j:j + 1],
                   nsin[:, 10 * lay + q:10 * lay + q + 1])
            yield
        for q in range(NQ):
            if q < RX_STD[lay]:
                jj = 10 * lay + q
                rot_rx_std(q, auxb[:, 1125 + jj:1126 + jj],
                           auxb[:, 1085 + jj:1086 + jj],
                           auxb[:, 1105 + jj:1106 + jj])
            else:
                j = 1044 + 10 * lay + q
                rot_rx_shear(q, auxb[:, j:j + 1], auxb[:, j + 20:j + 21])
            yield
        if lay == 0:
            yield from chain()

    # ---- probs + folded-head weighted reduce
    Pr, Pi = T[:, 0:NSTATE], T[:, NSTATE:]
    nc.scalar.square(T[:, :], S[:, :])  # both planes in one ACT op
    nc.vector.tensor_add(out=Pr, in0=Pr, in1=Pi)
    # (P*1.0) elementwise-mult W, accum_out = sum -> raw logit. (TTR compiles
    # but faults at runtime in this environment; STT+accum_out is equivalent.)
    nc.vector.scalar_tensor_tensor(
        Pi, Pr, 1.0, auxb[:, 0:NSTATE], Alu.mult, Alu.mult,
        accum_out=res[:, :])
    nc.sync.dma_start(out=out_d[t], in_=res[:, :])


def build_module():
    from concourse import bacc, mybir, tile
    f32 = mybir.dt.float32
    f16 = mybir.dt.float16
    nc = bacc.Bacc(None, target_bir_lowering=False)
    trig_d = nc.dram_tensor("trig", [TILES, P, TCOLS], f16,
                            kind="ExternalInput")
    aux_d = nc.dram_tensor("aux", [1, AUXW], f32, kind="ExternalInput")
    out_d = nc.dram_tensor("out", [TILES, P, 1], f32, kind="ExternalOutput")
    with tile.TileContext(nc) as tc:
        with tc.tile_pool(name="main", bufs=1) as pool:
            # aux block (W | theta scalars) -> partition 0, then log2-double
            # across partitions with SBUF->SBUF DMAs.
            auxb = pool.tile([P, AUXW], f32, name="auxb")
            nc.sync.dma_start(out=auxb[0:1, :], in_=aux_d[:, :])
            p = 1
            while p < P:
                nc.sync.dma_start(out=auxb[p:2 * p, :], in_=auxb[0:p, :])
                p *= 2
            # NSETS shared buffer sets; tiles run in waves of NSETS so the
            # scheduler overlaps one tile's DVE combines with the other's
            # ACT stage while SBUF stays bounded.
            sets = []
            for s in range(NSETS):
                sets.append((
                    pool.tile([P, TCOLS], f16, name=f"u16_{s}"),
                    pool.tile([P, TCOLS], f32, name=f"sinb{s}"),
                    pool.tile([P, TCOLS], f32, name=f"cosb{s}"),
                    pool.tile([P, TCOLS], f32, name=f"au{s}"),
                    pool.tile([P, 20], f32, name=f"nsin{s}"),
                    pool.tile([P, 40], f32, name=f"vblk{s}"),
                    pool.tile([P, 2 * NSTATE], f32, name=f"A{s}"),
                    pool.tile([P, 2 * NSTATE], f32, name=f"B{s}"),
                    pool.tile([P, 2 * NSTATE], f32, name=f"scr{s}"),
                    pool.tile([P, 1], f32, name=f"res{s}"),
                ))
            for w in range(0, TILES, NSETS):
                gens = [_emit_tile_gen(nc, w + s, sets[s], trig_d, auxb,
                                       out_d)
                        for s in range(min(NSETS, TILES - w))]
                live = list(gens)
                while live:
                    nxt_live = []
                    for g in live:
                        try:
                            next(g)
                            nxt_live.append(g)
                        except StopIteration:
                            pass
                    live = nxt_live
    nc.compile()  # Bacc pipeline: splits >1-wait instrs into event semaphores
    return nc


# ------------------------------------------------------------------ runner
def _build_runner():
    """AOT-compile the single-core module once; one uploaded buffer/call.

    fast_dispatch_compile suppresses the bass effect (C++ no-token dispatch).
    The exec-path custom call needs only the ExternalInput operands (trig,
    aux, partition_id) — "out" returns via the XLA result buffer — and aux
    is device-resident across calls (re-put only when params change).
    """
    import jax
    from concourse import bass2jax

    nc = build_module()
    bass2jax.install_neuronx_cc_hook()

    in_names = ["trig", "aux"]
    out_names = ["out"]
    all_in = in_names + ["partition_id"]
    out_avals = [jax.core.ShapedArray((TILES, P, 1), np.float32)]
    assert nc.partition_id_tensor is not None
    assert nc.dbg_addr is None, "build_module must not enable debug"

    def _body(trig, aux):
        operands = [trig, aux, bass2jax.partition_id_tensor()]
        return tuple(bass2jax._bass_exec_p.bind(
            *operands, out_avals=tuple(out_avals),
            in_names=tuple(all_in), out_names=tuple(out_names),
            lowering_input_output_aliases=(), sim_require_finite=True,
            sim_require_nnan=True, nc=nc))

    dev = jax.devices()[0]

    def compile_fast():
        f = jax.jit(_body, keep_unused=True)
        return f.lower(
            jax.ShapeDtypeStruct((TILES, P, TCOLS), np.float16),
            jax.ShapeDtypeStruct((1, AUXW), np.float32)).compile()

    jf = bass2jax.fast_dispatch_compile(compile_fast)

    state = {}

    def dispatch(trig_g, aux, params_key):
        aux_hit = state.get("aux")
        if aux_hit is None or aux_hit[0] is not params_key:
            aux_dev = jax.device_put(aux.reshape(1, AUXW), dev)
            state["aux"] = (params_key, aux_dev)
        else:
            aux_dev = aux_hit[1]
        return np.asarray(jf(trig_g, aux_dev)[0])

    return nc, dispatch


def _get_runner():
    if "runner" not in _CACHE:
        try:
            _CACHE["runner"] = _build_runner()
        except Exception:
            _CACHE["runner"] = None  # fall back to stock spmd dispatch
    return _CACHE["runner"]


def _dispatch_fallback(trig_g, aux):
    """Stock run_bass_kernel_spmd on core 0 (slow path: re-jits every call)."""
    from concourse.bass_utils import run_bass_kernel_spmd
    if "nc" not in _CACHE:
        _CACHE["nc"] = build_module()
    res = run_bass_kernel_spmd(
        _CACHE["nc"], [{"trig": trig_g, "aux": aux.reshape(1, AUXW)}],
        core_ids=[0])
    return res.results[0]["out"]


def kernel(x, theta, enc_alpha_raw, enc_beta_raw, head_w, head_b, logit_scale):
    # If the caller hands jax device arrays, start all host copies before the
    # first blocking np.asarray so the fetches overlap (one round trip, not 7).
    args = [x, theta, enc_alpha_raw, enc_beta_raw, head_w, head_b, logit_scale]
    for a in args:
        try:
            a.copy_to_host_async()
        except AttributeError:
            pass
    x, theta, enc_alpha_raw, enc_beta_raw, head_w, head_b, logit_scale = (
        np.asarray(a) for a in args)
    trig, aux = host_precompute(x, theta, enc_alpha_raw, enc_beta_raw,
                                head_w, head_b, logit_scale)
    runner = _get_runner()
    if runner is not None:
        params_key = _CACHE["params"][0]
        out = runner[1](trig, aux, params_key)
    else:
        out = _dispatch_fallback(trig, aux)
    out = out.reshape(BATCH, 1)
    return np.clip(out, -LOGIT_CLAMP, LOGIT_CLAMP).astype(np.float32)


# revision 24
# speedup vs baseline: 1.1071x; 1.0601x over previous
"""Trainium2 Bass kernel: nn_BinaryCQV_End2End (batched 10-qubit circuit sim).

Self-contained: host precompute (trig tables, folded head weight vector),
Bass module builder, cached-AOT PJRT runner.

Wall-clock structure under axon (remote NeuronCores behind a tunnel whose
RTT dominates): one blocking sync per dispatch is the floor. Measured on
this tunnel, per-call cost = 1 RTT + ~0.3 ms/uploaded buffer + ~0.25 ms
per execute, so the runner minimizes RPC traffic per call:

- ONE core, ONE module execute per call (16 batch tiles run as serial
  waves of two inside the kernel; device exec is ~2 ms, far below RTT).
- ONE uploaded buffer per call: the 120 KiB f16 trig tensor. The aux
  block (folded W + theta scalars) lives on the device across calls
  (re-put only when params change); outputs return via the XLA result
  buffer, so no dummy "out" operand is shipped.
- AOT compile once via fast_dispatch_compile (C++ no-effect dispatch),
  reused for the process lifetime.

The device computes sin/cos on ACT via Sin(pi*u) / Sin(-pi*|u| + pi/2) —
the HW Sin table is only accurate on [-pi, pi] (7.5e-2 error at 3pi/2),
hence the |u| form. Inputs ship as u = enc half-angle / pi wrapped to
[-1,1] in f16: quantizing the angle (not the trig values) keeps each gate
exactly unitary, so f16 shipping costs less accuracy than f16 trig did.

Layout: batch on partitions (128/tile, 16 tiles on core 0 = 2048).
State: one (128, 2048) f32 SBUF buffer per tile slot = [real | imag],
qubit 0 = MSB of the 10-bit state index.

Block-0 RY+RX are folded host-side into per-qubit amplitude pairs
(v0, v1); the device builds the product state by 9 doubling expansions.
Per rotation gate (per-partition scalars c, s from the trig tile):
  TS : T = swap_q(S) * s           (RX additionally swaps the r/i planes)
  STT: S = (S * c) -/+ T
CNOT chains run as triple-folds (8 strided copies of 256 each).
Block-2's trailing CNOT chain, head (incl. bias via sum(p)=1), logit
scale, and the RX tan-shear cos^2 factors fold into W on the host; the
device ends with probs + one weighted reduce. Final clamp is host-side.
"""
import numpy as np

NQ = 10
NSTATE = 1 << NQ  # 1024
INPUT_DIM = 49
ENC_LAMBDA = float(np.pi)
TILES = 16          # batch tiles, all on core 0
NSETS = 2           # concurrent SBUF buffer sets (tiles run in waves)
P = 128             # partitions
BATCH = TILES * P
TCOLS = 30          # per-sample cols (f16): u = half-angle/pi, layers 0|1|2
AUXW = 1024 + 10 + 10 + 20 + 20 + 1 + 60  # W|c2|ns2|tan|ntan|pi/2|s|ns|c
# theta-RX gates in "standard" form (4 ACT TS + 1 full-state DVE STT) instead
# of the tan-shear (4 DVE STT): balances ACT/DVE instruction issue, which is
# the exec bottleneck. Per layer: gates q < RX_STD[layer] are standard.
RX_STD = (4, 3)
LOGIT_SCALE_MIN, LOGIT_SCALE_MAX = 0.5, 80.0
LOGIT_CLAMP = 30.0

_CACHE = {}


# ---------------------------------------------------------------- host side
def _softplus(x):
    return np.log1p(np.exp(-np.abs(x))) + np.maximum(x, 0.0)


def _param_tables(theta, enc_alpha_raw, enc_beta_raw, head_w, head_b,
                  logit_scale):
    """x-independent tables: affine (A, B) for the angle transform and the
    aux vector. Memoized on exact input bytes — recomputed on any change."""
    key = tuple(np.asarray(v).tobytes() for v in
                (theta, enc_alpha_raw, enc_beta_raw, head_w, head_b,
                 logit_scale))
    hit = _CACHE.get("params")
    if hit is not None and hit[0] == key:
        return hit
    alpha = (_softplus(np.asarray(enc_alpha_raw, np.float64)) + 1e-6)[:3 * NQ]
    beta = np.tanh(np.asarray(enc_beta_raw, np.float64))[:3 * NQ]
    # u = h/pi = 0.5*(alpha*x + beta); ENC features are 0..29 for the 3
    # layers of 10 ((b*NQ+q) % INPUT_DIM).
    A = (0.5 * alpha).astype(np.float32)
    Bc = (0.5 * beta).astype(np.float32)
    th_h = 0.5 * np.asarray(theta, np.float64)  # (30,)
    c2, s2 = np.cos(th_h[:NQ]), np.sin(th_h[:NQ])

    tn = np.tan(th_h[NQ:3 * NQ])  # (20,)
    sx, cx = np.sin(th_h[NQ:3 * NQ]), np.cos(th_h[NQ:3 * NQ])
    aux = np.empty(AUXW, np.float32)
    aux[1024:1034] = c2
    aux[1034:1044] = -s2
    aux[1044:1064] = tn
    aux[1064:1084] = -tn
    aux[1084] = 0.5 * np.pi
    aux[1085:1105] = sx
    aux[1105:1125] = -sx
    aux[1125:1145] = cx

    # folded observable: W[i] = scale*(sum_q hw_q * z_q(perm(i)) + bias),
    # with block-2's CNOT chain as the bit permutation and the RX shears'
    # dropped cos factors folded in ((prod cos)^2 on probs).
    i = np.arange(NSTATE)
    W0 = np.zeros(NSTATE, np.float64)
    hw = np.asarray(head_w, np.float64)
    for q in range(NQ):
        W0 += hw[0, q] * (1.0 - 2.0 * ((i >> (NQ - 1 - q)) & 1))
    bits = (i[:, None] >> (NQ - 1 - np.arange(NQ))[None, :]) & 1
    nb = bits.copy()
    for c in range(NQ - 1):
        nb[:, c + 1] ^= nb[:, c]
    Pperm = (nb * (1 << (NQ - 1 - np.arange(NQ)))[None, :]).sum(1)
    scale = float(np.clip(np.asarray(logit_scale, np.float64),
                          LOGIT_SCALE_MIN, LOGIT_SCALE_MAX))
    W = scale * (W0[Pperm] + float(np.asarray(head_b).ravel()[0]))
    # cos^2 fold only over the SHEARED RX gates (standard-form gates apply
    # their cos factor on-device).
    sheared = [NQ + 10 * lay + q for lay in (0, 1)
               for q in range(RX_STD[lay], NQ)]
    W = W * float(np.prod(np.cos(th_h[sheared])) ** 2)
    aux[0:1024] = W.astype(np.float32)
    hit = (key, A, Bc, aux)
    _CACHE["params"] = hit
    return hit


def host_precompute(x, theta, enc_alpha_raw, enc_beta_raw, head_w, head_b,
                    logit_scale):
    """u (B, 30) f16 and aux (AUXW,) f32 [W | c2 | -s2 | tan | -tan | pi/2]."""
    key, A, Bc, aux = _param_tables(theta, enc_alpha_raw, enc_beta_raw,
                                    head_w, head_b, logit_scale)
    # Memoize the x-dependent table on exact x content (the device kernel
    # still runs in full every call; this only skips redundant host math).
    hit = _CACHE.get("trig")
    if hit is not None and hit[0] is key and np.array_equal(hit[1], x):
        return hit[2], aux
    buf = _CACHE.get("hostbuf")
    if buf is None:
        buf = (np.empty((BATCH, TCOLS), np.float32),
               np.empty((BATCH, TCOLS), np.float32),
               np.empty((TILES, P, TCOLS), np.float16))
        _CACHE["hostbuf"] = buf
    tmp, w, trig = buf
    # wrapped normalized half-angle: v = A*x + B; u = v - 2*rint(v/2) in
    # [-1, 1] (sin/cos of pi*u are 2-periodic in u, so the rint branch at
    # half-integers is immaterial). rint is ~100x faster than np.mod.
    np.multiply(np.asarray(x[:, :3 * NQ], np.float32), A[None, :], out=tmp)
    tmp += Bc[None, :]
    np.multiply(tmp, 0.5, out=w)
    np.rint(w, out=w)
    w *= 2.0
    tmp -= w
    trig.reshape(BATCH, TCOLS)[...] = tmp
    _CACHE["trig"] = (key, np.array(x), trig)
    return trig, aux


# ------------------------------------------------------------- device build
class _Sched:
    def __init__(self, nc):
        self.nc = nc
        self.act, self.dve, self.pool = nc.scalar, nc.vector, nc.gpsimd
        self._cp = 0

    # out = in0 * scalar  (scalar: AP (128,1) or float)
    def ts(self, out, in0, scalar):
        self.act.mul(out, in0, scalar)

    # out = (in0 * scalar) op1 in1 — DVE only (Pool HW rejects TensorScalarPtr)
    def stt(self, out, in0, scalar, in1, sub):
        from concourse import mybir
        self.dve.scalar_tensor_tensor(
            out, in0, scalar, in1, mybir.AluOpType.mult,
            mybir.AluOpType.subtract if sub else mybir.AluOpType.add)

    def cp(self, out, in0):
        pick = [self.act, self.pool][self._cp % 2]
        self._cp += 1
        if pick is self.act:
            pick.copy(out, in0)
        else:
            pick.tensor_copy(out=out, in_=in0)


def _emit_tile_gen(nc, t, bufs, trig_d, auxb, out_d):
    from concourse import mybir
    Alu = mybir.AluOpType
    sch = _Sched(nc)
    PI = float(np.pi)
    Sin, Abs = (mybir.ActivationFunctionType.Sin,
                mybir.ActivationFunctionType.Abs)
    u16, sinb, cosb, au, nsin, vblk, A, B, scr, res = bufs
    nc.sync.dma_start(out=u16[:, :], in_=trig_d[t])
    nc.scalar.activation(sinb[:, :], u16[:, :], Sin, bias=0.0, scale=PI)
    nc.scalar.activation(nsin[:, :], u16[:, 10:30], Sin, bias=0.0, scale=-PI)
    nc.scalar.activation(au[:, :], u16[:, :], Abs)
    # cos(pi*u) = sin(pi/2 - pi*|u|); argument stays in [-pi/2, pi/2] where
    # the HW Sin table is accurate (it degrades badly beyond +-pi).
    nc.scalar.activation(cosb[:, :], au[:, :], Sin,
                         bias=auxb[:, 1084:1085], scale=-PI)

    # ---- block-0 fold: v0 = c2*ca - i*s2*sa ; v1 = c2*sa - i*s2*ca
    c2, ns2 = auxb[:, 1024:1034], auxb[:, 1034:1044]
    ca, sa = cosb[:, 0:10], sinb[:, 0:10]
    nc.vector.tensor_mul(out=vblk[:, 0:10], in0=ca, in1=c2)     # v0r
    nc.vector.tensor_mul(out=vblk[:, 10:20], in0=sa, in1=c2)    # v1r
    nc.vector.tensor_mul(out=vblk[:, 20:30], in0=sa, in1=ns2)   # v0i
    nc.vector.tensor_mul(out=vblk[:, 30:40], in0=ca, in1=ns2)   # v1i
    yield

    # ---- product state with block-0's CNOT chain FOLDED IN.
    # The chain i -> prefix-XOR(i) makes the post-chain state a generalized
    # product: S[b0..b9] = v0[b0] * prod_q v_q[b_q ^ b_{q-1}]. Level q writes
    # pairs (b_{q-1}, b_q) of equal XOR-parity pcls with multiplier v_q[pcls];
    # those positions 4*hi + 2*bp + (bp^pcls) are a diagonal-stride AP
    # ([4,n/2],[3,2] resp. offset+1 [4,n/2],[1,2]), so the fold is free.
    from concourse.ap import AP as _AP

    def vcol(q, b, im):
        j = 20 * im + 10 * b + q
        return vblk[:, j:j + 1]

    def hb(ap_flat, h):  # contiguous [h, 2] view of a flat length-2h slice
        return ap_flat.rearrange("p (h b) -> p h b", h=h, b=2)

    def diag(buf, plane, n, pcls):  # out view, level writing 2n amps
        base = buf[:, 0:1]
        off = base.offset + NSTATE * plane + pcls
        st = 3 if pcls == 0 else 1
        return _AP(tensor=base.tensor, offset=off,
                   ap=[list(base.ap)[0], [4, n // 2], [st, 2]])

    cur, nxt = A, B
    for q in range(1, NQ):
        n = 1 << q
        if q == 1:  # level-1 amps are qubit-0's v pair, read strided
            ar, ai = hb(vblk[:, 0:11:10], 1), hb(vblk[:, 20:31:10], 1)
        else:
            ar = hb(cur[:, 0:n], n // 2)
            ai = hb(cur[:, NSTATE:NSTATE + n], n // 2)
        for pcls in (0, 1):
            vr, vi = vcol(q, pcls, 0), vcol(q, pcls, 1)
            out_r = diag(nxt, 0, n, pcls)
            out_i = diag(nxt, 1, n, pcls)
            sr = hb(scr[:, 512 * pcls:512 * pcls + n], n // 2)
            si = hb(scr[:, NSTATE + 512 * pcls:NSTATE + 512 * pcls + n],
                    n // 2)
            sch.ts(sr, ai, vi)
            sch.stt(out_r, ar, vr, sr, sub=True)       # ar*vr - ai*vi
            sch.ts(si, ai, vr)
            sch.stt(out_i, ar, vi, si, sub=False)      # ar*vi + ai*vr
        cur, nxt = nxt, cur
        yield
    S, T = cur, nxt

    # HW APs allow at most 3 free dims. The plane dim (stride 1024) folds with
    # the hi dim (count 2^q, stride 2^(10-q)) into one "ph" dim of 2^(q+1).
    def view3(buf, q):
        return buf[:, :].rearrange("p (ph b lo) -> p ph b lo",
                                   ph=1 << (q + 1), b=2, lo=1 << (NQ - 1 - q))

    def plane_view3(buf, plane, q):  # one plane, [hi, b, lo]
        pl = buf[:, NSTATE * plane:NSTATE * (plane + 1)]
        return pl.rearrange("p (hi b lo) -> p hi b lo",
                            hi=1 << q, b=2, lo=1 << (NQ - 1 - q))

    def rot_ry(q, ccol, scol, nscol):
        # ENC RY: T[bit0] = -s*s1 ; T[bit1] = +s*s0 (ACT), then one in-place
        # 2048-elem DVE STT: S = S*c + T.
        Sv, Tv = view3(S, q), view3(T, q)
        sch.ts(Tv[:, :, 0, :], Sv[:, :, 1, :], nscol)
        sch.ts(Tv[:, :, 1, :], Sv[:, :, 0, :], scol)
        sch.stt(S[:, :], S[:, :], ccol, T[:, :], sub=False)

    def rot_rx_std(q, ccol, scol, nscol):
        # standard RX: T_r = s*swap(S_i); T_i = -s*swap(S_r) (4 ACT TS,
        # b-split 2-dim views), then ONE full-state DVE STT: S = c*S + T.
        Tv = [plane_view3(T, p_, q) for p_ in (0, 1)]
        Sv = [plane_view3(S, p_, q) for p_ in (0, 1)]
        for b in (0, 1):
            sch.ts(Tv[0][:, :, b, :], Sv[1][:, :, 1 - b, :], scol)
            sch.ts(Tv[1][:, :, b, :], Sv[0][:, :, 1 - b, :], nscol)
        sch.stt(S[:, :], S[:, :], ccol, T[:, :], sub=False)

    def rot_rx_shear(q, tcol, ntcol):
        # theta-RX tan-shear: T_r = t*swap(S_i) + S_r ; T_i = -t*swap(S_r)+S_i
        # cos factor folded into W host-side. Buffer swap. STT allows only 2
        # free dims -> split by (plane, bit): 4 DVE STTs of 512.
        nonlocal S, T
        Tv = [plane_view3(T, p_, q) for p_ in (0, 1)]
        Sv = [plane_view3(S, p_, q) for p_ in (0, 1)]
        for b in (0, 1):
            sch.stt(Tv[0][:, :, b, :], Sv[1][:, :, 1 - b, :], tcol,
                    Sv[0][:, :, b, :], sub=False)
            sch.stt(Tv[1][:, :, b, :], Sv[0][:, :, 1 - b, :], ntcol,
                    Sv[1][:, :, b, :], sub=False)
        S, T = T, S

    def cnot_triple(c):
        # CNOT(c,c+1);CNOT(c+1,c+2);CNOT(c+2,c+3):
        # out[B0,B1,B2,B3] = in[B0, B1^B0, B2^B1, B3^B2]. 8 copies of 256.
        nonlocal S, T
        def v(buf):
            return buf[:, :].rearrange(
                "p (ph bc b1 b2 b3 lo) -> p ph bc b1 b2 b3 lo",
                ph=1 << (c + 1), bc=2, b1=2, b2=2, b3=2, lo=1 << (NQ - 4 - c))
        Sv, Tv = v(S), v(T)
        for B0 in (0, 1):
            for B1 in (0, 1):
                for B2 in (0, 1):
                    outv = Tv[:, :, B0, B1, B2, :, :]
                    if B2:
                        inv = Sv[:, :, B0, B1 ^ B0, B2 ^ B1, ::-1, :]
                    else:
                        inv = Sv[:, :, B0, B1 ^ B0, B2 ^ B1, :, :]
                    sch.cp(outv, inv)
        S, T = T, S

    def chain():
        for c in (0, 3, 6):
            cnot_triple(c)
            yield

    for lay in (0, 1):
        for q in range(NQ):
            j = 10 + 10 * lay + q
            rot_ry(q, cosb[:, j:j + 1], sinb[:, j:j + 1],
                   nsin[:, 10 * lay + q:10 * lay + q + 1])
            yield
        for q in range(NQ):
            if q < RX_STD[lay]:
                jj = 10 * lay + q
                rot_rx_std(q, auxb[:, 1125 + jj:1126 + jj],
                           auxb[:, 1085 + jj:1086 + jj],
                           auxb[:, 1105 + jj:1106 + jj])
            else:
                j = 1044 + 10 * lay + q
                rot_rx_shear(q, auxb[:, j:j + 1], auxb[:, j + 20:j + 21])
            yield
        if lay == 0:
            yield from chain()

    # ---- probs + folded-head weighted reduce
    Pr, Pi = T[:, 0:NSTATE], T[:, NSTATE:]
    nc.scalar.square(T[:, :], S[:, :])  # both planes in one ACT op
    nc.vector.tensor_add(out=Pr, in0=Pr, in1=Pi)
    # (P*1.0) elementwise-mult W, accum_out = sum -> raw logit. (TTR compiles
    # but faults at runtime in this environment; STT+accum_out is equivalent.)
    nc.vector.scalar_tensor_tensor(
        Pi, Pr, 1.0, auxb[:, 0:NSTATE], Alu.mult, Alu.mult,
        accum_out=res[:, :])
    nc.sync.dma_start(out=out_d[t], in_=res[:, :])


def build_module():
    from concourse import bacc, mybir, tile
    f32 = mybir.dt.float32
    f16 = mybir.dt.float16
    nc = bacc.Bacc(None, target_bir_lowering=False)
    trig_d = nc.dram_tensor("trig", [TILES, P, TCOLS], f16,
                            kind="ExternalInput")
    aux_d = nc.dram_tensor("aux", [1, AUXW], f32, kind="ExternalInput")
    out_d = nc.dram_tensor("out", [TILES, P, 1], f32, kind="ExternalOutput")
    with tile.TileContext(nc) as tc:
        with tc.tile_pool(name="main", bufs=1) as pool:
            # aux block (W | theta scalars) -> partition 0, then log2-double
            # across partitions with SBUF->SBUF DMAs.
            auxb = pool.tile([P, AUXW], f32, name="auxb")
            nc.sync.dma_start(out=auxb[0:1, :], in_=aux_d[:, :])
            p = 1
            while p < P:
                nc.sync.dma_start(out=auxb[p:2 * p, :], in_=auxb[0:p, :])
                p *= 2
            # NSETS shared buffer sets; tiles run in waves of NSETS so the
            # scheduler overlaps one tile's DVE combines with the other's
            # ACT stage while SBUF stays bounded.
            sets = []
            for s in range(NSETS):
                sets.append((
                    pool.tile([P, TCOLS], f16, name=f"u16_{s}"),
                    pool.tile([P, TCOLS], f32, name=f"sinb{s}"),
                    pool.tile([P, TCOLS], f32, name=f"cosb{s}"),
                    pool.tile([P, TCOLS], f32, name=f"au{s}"),
                    pool.tile([P, 20], f32, name=f"nsin{s}"),
                    pool.tile([P, 40], f32, name=f"vblk{s}"),
                    pool.tile([P, 2 * NSTATE], f32, name=f"A{s}"),
                    pool.tile([P, 2 * NSTATE], f32, name=f"B{s}"),
                    pool.tile([P, 2 * NSTATE], f32, name=f"scr{s}"),
                    pool.tile([P, 1], f32, name=f"res{s}"),
                ))
            for w in range(0, TILES, NSETS):
                gens = [_emit_tile_gen(nc, w + s, sets[s], trig_d, auxb,
                                       out_d)
                        for s in range(min(NSETS, TILES - w))]
                live = list(gens)
                while live:
                    nxt_live = []
                    for g in live:
                        try:
                            next(g)
                            nxt_live.append(g)
                        except StopIteration:
                            pass
                    live = nxt_live
    nc.compile()  # Bacc pipeline: splits >1-wait instrs into event semaphores
    return nc


# ------------------------------------------------------------------ runner
def _build_runner():
    """AOT-compile the single-core module once; one uploaded buffer/call.

    fast_dispatch_compile suppresses the bass effect (C++ no-token dispatch).
    The exec-path custom call needs only the ExternalInput operands (trig,
    aux, partition_id) — "out" returns via the XLA result buffer — and aux
    is device-resident across calls (re-put only when params change).
    """
    import jax
    from concourse import bass2jax

    nc = build_module()
    bass2jax.install_neuronx_cc_hook()

    in_names = ["trig", "aux"]
    out_names = ["out"]
    all_in = in_names + ["partition_id"]
    out_avals = [jax.core.ShapedArray((TILES, P, 1), np.float32)]
    assert nc.partition_id_tensor is not None
    assert nc.dbg_addr is None, "build_module must not enable debug"

    def _body(trig, aux):
        operands = [trig, aux, bass2jax.partition_id_tensor()]
        return tuple(bass2jax._bass_exec_p.bind(
            *operands, out_avals=tuple(out_avals),
            in_names=tuple(all_in), out_names=tuple(out_names),
            lowering_input_output_aliases=(), sim_require_finite=True,
            sim_require_nnan=True, nc=nc))

    dev = jax.devices()[0]

    def compile_fast():
        f = jax.jit(_body, keep_unused=True)
        return f.lower(
            jax.ShapeDtypeStruct((TILES, P, TCOLS), np.float16),
            jax.ShapeDtypeStruct((1, AUXW), np.float32)).compile()

    jf = bass2jax.fast_dispatch_compile(compile_fast)

    state = {}

    def dispatch(trig_g, aux, params_key):
        aux_hit = state.get("aux")
        if aux_hit is None or aux_hit[0] is not params_key:
            aux_dev = jax.device_put(aux.reshape(1, AUXW), dev)
            state["aux"] = (params_key, aux_dev)
        else:
            aux_dev = aux_hit[1]
        return np.asarray(jf(trig_g, aux_dev)[0])

    return nc, dispatch


def _get_runner():
    if "runner" not in _CACHE:
        try:
            _CACHE["runner"] = _build_runner()
        except Exception:
            _CACHE["runner"] = None  # fall back to stock spmd dispatch
    return _CACHE["runner"]


def _dispatch_fallback(trig_g, aux):
    """Stock run_bass_kernel_spmd on core 0 (slow path: re-jits every call)."""
    from concourse.bass_utils import run_bass_kernel_spmd
    if "nc" not in _CACHE:
        _CACHE["nc"] = build_module()
    res = run_bass_kernel_spmd(
        _CACHE["nc"], [{"trig": trig_g, "aux": aux.reshape(1, AUXW)}],
        core_ids=[0])
    return res.results[0]["out"]


def kernel(x, theta, enc_alpha_raw, enc_beta_raw, head_w, head_b, logit_scale):
    # If the caller hands jax device arrays, start all host copies before the
    # first blocking np.asarray so the fetches overlap (one round trip, not 7).
    args = [x, theta, enc_alpha_raw, enc_beta_raw, head_w, head_b, logit_scale]
    for a in args:
        try:
            a.copy_to_host_async()
        except AttributeError:
            pass
    x, theta, enc_alpha_raw, enc_beta_raw, head_w, head_b, logit_scale = (
        np.asarray(a) for a in args)
    trig, aux = host_precompute(x, theta, enc_alpha_raw, enc_beta_raw,
                                head_w, head_b, logit_scale)
    runner = _get_runner()
    if runner is not None:
        params_key = _CACHE["params"][0]
        out = runner[1](trig, aux, params_key)
    else:
        out = _dispatch_fallback(trig, aux)
    out = out.reshape(BATCH, 1)
    return np.clip(out, -LOGIT_CLAMP, LOGIT_CLAMP).astype(np.float32)
